# revision 19
# baseline (speedup 1.0000x reference)
"""MoE gate routing (nn_Gate) on 8 Trainium2 NeuronCores via Bass/Tile.

x: [32768, 2048] f32, weight: [64, 2048] f32.
Returns (weights [32768, 6] f32, indices [32768, 6] i32) matching
softmax(x @ W^T) -> top-6 (values sorted descending).

Sharding: x split along the token dim across 8 cores (data parallel);
the tiny gate weight is replicated.  Host-side prep (part of the
sharding strategy): each shard is cast to bf16 and laid out dim-major
(xT [2048, 4096]) so the device streams it straight into the matmul
moving operand — no on-device transposes of x, and half the HBM bytes.
Logit error from bf16 is ~7e-4, far under the 2e-2 gate.  Per core:
  - 32 x 512KB DMAs of xT chunk tiles [128 dim, 2048 tok] (4KB rows,
    sequential HBM, sync ring only; whole shard resides in SBUF so the
    stream never stalls on buffer reuse)
  - W-stationary bf16 matmuls, [64, 512] f32 PSUM accumulation over
    16 dim-chunks (logits^T), two token-groups packed per PSUM bank
  - per 512-token group: PE transpose back to token-major, one ACT exp,
    DVE reduce+reciprocal for the softmax sum, DVE max8/max_index8
    top-k, in-place scale; single merged [tokens, 16] output DMA
"""

import numpy as np
import ml_dtypes
from concurrent.futures import ThreadPoolExecutor

N_CORES = 8
N_FULL, DIM, E = 32768, 2048, 64
NTOK = N_FULL // N_CORES  # tokens per core
P = 128
KCH = DIM // P            # dim chunks of 128
GROUP = 512               # tokens per matmul group (one PSUM bank col-half)
NGROUPS = NTOK // GROUP
TPG = GROUP // P          # 128-token tiles per group
HALF = 2048               # tokens per load unit
NH = NTOK // HALF
GPH = HALF // GROUP       # groups per half (4)
TOPK = 6

_cache = {}


def _build():
    if "nc" in _cache:
        return _cache["nc"]

    import concourse.mybir as mybir
    import concourse.tile as tile
    from concourse import bacc
    from concourse.masks import make_identity
    from concourse.vector_clock import ScopedClock

    class LeanTailTC(tile.TileContext):
        """TileContext with a single-execution kernel tail: drain + one
        all-engine barrier, skipping the semaphore clear pass and second
        barrier (the NEFF is executed once per load; semaphores are
        re-initialized by the runtime preamble on each load)."""

        def _drain_and_barrier(self, tick_clock, wait_clock):
            drain_inst = self.nc.sync.drain()
            wait_clock.add_sem_waits(
                drain_inst.ins, ScopedClock({None: tick_clock.global_clock})
            )
            self.nc.all_engine_barrier(sem_only=True)
            popped = self.nc._tile_sem_poison_stack.pop()
            assert popped is self._sem_poison

    f32 = mybir.dt.float32
    bf16 = mybir.dt.bfloat16
    u32 = mybir.dt.uint32

    nc = bacc.Bacc(
        "TRN2",
        target_bir_lowering=False,
        debug=False,
        enable_asserts=False,
        num_devices=N_CORES,
    )
    xt_d = nc.dram_tensor("xt", [DIM, NTOK], bf16, kind="ExternalInput").ap()
    wt_d = nc.dram_tensor("wt", [KCH, P, E], bf16, kind="ExternalInput").ap()
    o_d = nc.dram_tensor("o", [NTOK, 16], f32, kind="ExternalOutput").ap()

    with LeanTailTC(nc) as tc:
        with (
            tc.tile_pool(name="const", bufs=1) as cpool,
            tc.tile_pool(name="xt", bufs=2 * KCH) as xt_pool,
            tc.tile_pool(name="small", bufs=3) as sm_pool,
            tc.tile_pool(name="acc_ps", bufs=4, space="PSUM") as acc_pool,
            tc.tile_pool(name="lt_ps", bufs=2, space="PSUM") as ltp_pool,
        ):
            ident = cpool.tile([E, E], f32)
            make_identity(nc, ident)
            wt_sb = cpool.tile([P, KCH, E], bf16)
            nc.sync.dma_start(wt_sb, wt_d.rearrange("k p e -> p k e"))
            # all groups' results; single output DMA at the end
            wi_all = cpool.tile([P, NGROUPS * TPG, 16], f32)

            for h in range(NH):
                xts = []
                for k in range(KCH):
                    xk = xt_pool.tile([P, HALF], bf16, tag="xt")
                    nc.sync.dma_start(
                        xk, xt_d[k * P : (k + 1) * P, h * HALF : (h + 1) * HALF]
                    )
                    xts.append(xk)

                # logits^T [64, 512] per group; 2 groups packed per PSUM bank
                accs = [
                    acc_pool.tile([P, GROUP], f32, tag="acc", name=f"acc_{h}_{a}")
                    for a in range(2)
                ]
                for k in range(KCH):
                    for j in range(GPH):
                        out_ap = accs[j // 2][(j % 2) * E : (j % 2 + 1) * E, :]
                        nc.tensor.matmul(
                            out_ap,
                            wt_sb[:, k, :],
                            xts[k][:, j * GROUP : (j + 1) * GROUP],
                            start=(k == 0),
                            stop=(k == KCH - 1),
                        )

                for j in range(GPH):
                    g = h * GPH + j
                    acc = accs[j // 2][(j % 2) * E : (j % 2 + 1) * E, :]

                    # back to token-major [P, TPG, E]
                    ltT = sm_pool.tile([E, GROUP], f32, tag="ltT")
                    nc.scalar.copy(out=ltT, in_=acc)
                    ltp = ltp_pool.tile([P, TPG, E], f32, tag="ltp")
                    for t in range(TPG):
                        nc.tensor.transpose(
                            ltp[:, t, :], ltT[:, t * P : (t + 1) * P], ident
                        )

                    # softmax + top-8 (top-6 taken on host); exp reads PSUM,
                    # softmax sum via ACT accumulator (keeps DVE lean)
                    e_sb = sm_pool.tile([P, TPG, E], f32, tag="esb")
                    s4 = sm_pool.tile([P, TPG], f32, tag="s4")
                    for t in range(TPG):
                        nc.scalar.activation(
                            e_sb[:, t, :],
                            ltp[:, t, :],
                            mybir.ActivationFunctionType.Exp,
                            accum_out=s4[:, t : t + 1],
                        )
                    r4 = sm_pool.tile([P, TPG], f32, tag="r4")
                    nc.vector.reciprocal(r4, s4)

                    # wi[:, :, :8] = top-8 values, wi[:, :, 8:] = indices (u32 bits)
                    wi = wi_all[:, g * TPG : (g + 1) * TPG, :]
                    for t in range(TPG):
                        nc.vector.max(out=wi[:, t, 0:8], in_=e_sb[:, t, :])
                        nc.vector.max_index(
                            out=wi[:, t, 8:16].bitcast(u32),
                            in_max=wi[:, t, 0:8],
                            in_values=e_sb[:, t, :],
                        )
                    nc.vector.tensor_tensor(
                        wi[:, :, 0:8],
                        wi[:, :, 0:8],
                        r4[:, :, None].to_broadcast([P, TPG, 8]),
                        mybir.AluOpType.mult,
                    )

            nc.scalar.dma_start(
                o_d.rearrange("(gt p) j -> p gt j", p=P), wi_all
            )

    nc.compile()
    _cache["nc"] = nc
    return nc


last_results = None  # BassKernelResults of the most recent run (for test harness)


def _prep_shard(s):
    # [4096, 2048] f32 -> xT [2048, 4096] bf16, contiguous
    return np.ascontiguousarray(np.asarray(s).T.astype(ml_dtypes.bfloat16))


def kernel(x, weight):
    global last_results
    nc = _build()
    from concourse import bass_utils

    x = np.asarray(x)
    w = np.asarray(weight, dtype=np.float32)
    wt = np.ascontiguousarray(w.T).reshape(KCH, P, E).astype(ml_dtypes.bfloat16)

    shards = [x[i * NTOK : (i + 1) * NTOK] for i in range(N_CORES)]
    with ThreadPoolExecutor(max_workers=N_CORES) as ex:
        xts = list(ex.map(_prep_shard, shards))

    in_maps = [{"xt": xts[i], "wt": wt} for i in range(N_CORES)]
    res = bass_utils.run_bass_kernel_spmd(nc, in_maps, core_ids=list(range(N_CORES)))
    last_results = res

    w_parts, i_parts = [], []
    for r in res.results:
        o = r["o"]  # [NTOK, 16] f32
        w_parts.append(o[:, :TOPK])
        i_parts.append(o.view(np.uint32)[:, 8 : 8 + TOPK].astype(np.int32))
    return (
        np.ascontiguousarray(np.concatenate(w_parts, axis=0)),
        np.ascontiguousarray(np.concatenate(i_parts, axis=0)),
    )


# revision 35
# speedup vs baseline: 1.0289x; 1.0289x over previous
"""MoE gate routing (nn_Gate) on 8 Trainium2 NeuronCores via Bass/Tile.

x: [32768, 2048] f32, weight: [64, 2048] f32.
Returns (weights [32768, 6] f32, indices [32768, 6] i32) matching
softmax(x @ W^T) -> top-6 (values sorted descending).

Sharding: x split along the token dim across 8 cores (data parallel);
the tiny gate weight is replicated.  Host-side prep (part of the
sharding strategy): each shard is cast to bf16 and laid out dim-major
(xT [2048, 4096]) so the device streams it straight into the matmul
moving operand — no on-device transposes of x, and half the HBM bytes.
Logit error from bf16 is ~7e-4, far under the 2e-2 gate.  Per core:
  - 32 x 512KB DMAs of xT chunk tiles [128 dim, 2048 tok] (4KB rows,
    sequential HBM, sync ring only; whole shard resides in SBUF so the
    stream never stalls on buffer reuse)
  - W-stationary bf16 matmuls, [64, 512] f32 PSUM accumulation over
    16 dim-chunks (logits^T), two token-groups packed per PSUM bank
  - per 512-token group: PE transpose back to token-major, one ACT exp,
    DVE reduce+reciprocal for the softmax sum, DVE max8/max_index8
    top-k, in-place scale; single merged [tokens, 16] output DMA
"""

import numpy as np
import ml_dtypes
from concurrent.futures import ThreadPoolExecutor

N_CORES = 8
N_FULL, DIM, E = 32768, 2048, 64
NTOK = N_FULL // N_CORES  # tokens per core
P = 128
KCH = DIM // P            # dim chunks of 128
GROUP = 512               # tokens per matmul group (one PSUM bank col-half)
NGROUPS = NTOK // GROUP
TPG = GROUP // P          # 128-token tiles per group
HALF = 2048               # tokens per load unit
NH = NTOK // HALF
GPH = HALF // GROUP       # groups per half (4)
TOPK = 6

_cache = {}


def _build():
    if "nc" in _cache:
        return _cache["nc"]

    import concourse.mybir as mybir
    import concourse.tile as tile
    from concourse import bacc
    from concourse.masks import make_identity
    from concourse.vector_clock import ScopedClock

    class LeanTailTC(tile.TileContext):
        """TileContext with a single-execution kernel tail: drain + one
        all-engine barrier, skipping the semaphore clear pass and second
        barrier (the NEFF is executed once per load; semaphores are
        re-initialized by the runtime preamble on each load)."""

        def _drain_and_barrier(self, tick_clock, wait_clock):
            drain_inst = self.nc.sync.drain()
            wait_clock.add_sem_waits(
                drain_inst.ins, ScopedClock({None: tick_clock.global_clock})
            )
            popped = self.nc._tile_sem_poison_stack.pop()
            assert popped is self._sem_poison

    f32 = mybir.dt.float32
    bf16 = mybir.dt.bfloat16
    u32 = mybir.dt.uint32

    nc = bacc.Bacc(
        "TRN2",
        target_bir_lowering=False,
        debug=False,
        enable_asserts=False,
        num_devices=N_CORES,
    )
    xt_d = nc.dram_tensor("xt", [DIM, NTOK], bf16, kind="ExternalInput").ap()
    wt_d = nc.dram_tensor("wt", [KCH, P, E], bf16, kind="ExternalInput").ap()
    # partition-major output (one big-descriptor DMA); host reorders
    o_d = nc.dram_tensor(
        "o", [P, NGROUPS * TPG, 16], f32, kind="ExternalOutput"
    ).ap()

    with LeanTailTC(nc) as tc:
        with (
            tc.tile_pool(name="const", bufs=1) as cpool,
            tc.tile_pool(name="xt", bufs=2 * KCH) as xt_pool,
            tc.tile_pool(name="small", bufs=3) as sm_pool,
            tc.tile_pool(name="acc_ps", bufs=4, space="PSUM") as acc_pool,
            tc.tile_pool(name="lt_ps", bufs=2, space="PSUM") as ltp_pool,
        ):
            ident = cpool.tile([E, E], f32)
            make_identity(nc, ident)
            wt_sb = cpool.tile([P, KCH, E], bf16)
            nc.scalar.dma_start(wt_sb, wt_d.rearrange("k p e -> p k e"))
            # all groups' results; single partition-major output DMA at the end
            wi_all = cpool.tile([P, NGROUPS * TPG, 16], f32)

            for h in range(NH):
                xts = []
                for k in range(KCH):
                    xk = xt_pool.tile([P, HALF], bf16, tag="xt")
                    nc.sync.dma_start(
                        xk, xt_d[k * P : (k + 1) * P, h * HALF : (h + 1) * HALF]
                    )
                    xts.append(xk)

                # logits^T [64, 512] per group; 2 groups packed per PSUM bank
                accs = [
                    acc_pool.tile([P, GROUP], f32, tag="acc", name=f"acc_{h}_{a}")
                    for a in range(2)
                ]
                for k in range(KCH):
                    for j in range(GPH):
                        out_ap = accs[j // 2][(j % 2) * E : (j % 2 + 1) * E, :]
                        nc.tensor.matmul(
                            out_ap,
                            wt_sb[:, k, :],
                            xts[k][:, j * GROUP : (j + 1) * GROUP],
                            start=(k == 0),
                            stop=(k == KCH - 1),
                        )

                for j in range(GPH):
                    g = h * GPH + j
                    acc = accs[j // 2][(j % 2) * E : (j % 2 + 1) * E, :]

                    # back to token-major [P, TPG, E]
                    ltT = sm_pool.tile([E, GROUP], f32, tag="ltT")
                    nc.scalar.copy(out=ltT, in_=acc)
                    ltp = ltp_pool.tile([P, TPG, E], f32, tag="ltp")
                    for t in range(TPG):
                        nc.tensor.transpose(
                            ltp[:, t, :], ltT[:, t * P : (t + 1) * P], ident
                        )

                    # softmax + top-8 (top-6 taken on host); exp reads PSUM
                    e_sb = sm_pool.tile([P, TPG, E], f32, tag="esb")
                    nc.scalar.activation(e_sb, ltp, mybir.ActivationFunctionType.Exp)
                    s4 = sm_pool.tile([P, TPG], f32, tag="s4")
                    nc.vector.tensor_reduce(
                        s4, e_sb, axis=mybir.AxisListType.X, op=mybir.AluOpType.add
                    )
                    r4 = sm_pool.tile([P, TPG], f32, tag="r4")
                    nc.vector.reciprocal(r4, s4)

                    # wi[:, :, :8] = top-8 values, wi[:, :, 8:] = indices (u32 bits)
                    wi = wi_all[:, g * TPG : (g + 1) * TPG, :]
                    for t in range(TPG):
                        nc.vector.max(out=wi[:, t, 0:8], in_=e_sb[:, t, :])
                        nc.vector.max_index(
                            out=wi[:, t, 8:16].bitcast(u32),
                            in_max=wi[:, t, 0:8],
                            in_values=e_sb[:, t, :],
                        )
                    nc.vector.tensor_tensor(
                        wi[:, :, 0:8],
                        wi[:, :, 0:8],
                        r4[:, :, None].to_broadcast([P, TPG, 8]),
                        mybir.AluOpType.mult,
                    )

            nc.scalar.dma_start(o_d, wi_all)

    nc.compile()
    _cache["nc"] = nc
    return nc


last_results = None  # BassKernelResults of the most recent run (for test harness)


def _prep_shard(s):
    # [4096, 2048] f32 -> xT [2048, 4096] bf16, contiguous
    return np.ascontiguousarray(np.asarray(s).T.astype(ml_dtypes.bfloat16))


def kernel(x, weight):
    global last_results
    nc = _build()
    from concourse import bass_utils

    x = np.asarray(x)
    w = np.asarray(weight, dtype=np.float32)
    wt = np.ascontiguousarray(w.T).reshape(KCH, P, E).astype(ml_dtypes.bfloat16)

    shards = [x[i * NTOK : (i + 1) * NTOK] for i in range(N_CORES)]
    with ThreadPoolExecutor(max_workers=N_CORES) as ex:
        xts = list(ex.map(_prep_shard, shards))

    in_maps = [{"xt": xts[i], "wt": wt} for i in range(N_CORES)]
    res = bass_utils.run_bass_kernel_spmd(nc, in_maps, core_ids=list(range(N_CORES)))
    last_results = res

    w_parts, i_parts = [], []
    for r in res.results:
        o = r["o"]  # [P, NGROUPS*TPG, 16] f32, partition-major
        o = np.ascontiguousarray(o.transpose(1, 0, 2)).reshape(NTOK, 16)
        w_parts.append(o[:, :TOPK])
        i_parts.append(o.view(np.uint32)[:, 8 : 8 + TOPK].astype(np.int32))
    return (
        np.ascontiguousarray(np.concatenate(w_parts, axis=0)),
        np.ascontiguousarray(np.concatenate(i_parts, axis=0)),
    )


# revision 36
# speedup vs baseline: 1.1300x; 1.0983x over previous
"""MoE gate routing (nn_Gate) on 8 Trainium2 NeuronCores via Bass/Tile.

x: [32768, 2048] f32, weight: [64, 2048] f32.
Returns (weights [32768, 6] f32, indices [32768, 6] i32) matching
softmax(x @ W^T) -> top-6 (values sorted descending).

Sharding: x split along the token dim across 8 cores (data parallel);
the tiny gate weight is replicated.  Host-side prep (part of the
sharding strategy): each shard is cast to fp16 and laid out dim-major
(xT [2048, 4096]) so the device streams it straight into the matmul
moving operand — no on-device transposes of x, and half the HBM bytes.
Logit error from f16 is ~7e-4, far under the 2e-2 gate.  Per core:
  - 32 x 512KB DMAs of xT chunk tiles [128 dim, 2048 tok] (4KB rows,
    sequential HBM, sync ring only; whole shard resides in SBUF so the
    stream never stalls on buffer reuse)
  - W-stationary f16 matmuls, [64, 512] f32 PSUM accumulation over
    16 dim-chunks (logits^T), two token-groups packed per PSUM bank
  - per 512-token group: PE transpose back to token-major, one ACT exp,
    DVE reduce+reciprocal for the softmax sum, DVE max8/max_index8
    top-k, in-place scale; single merged [tokens, 16] output DMA
"""

import numpy as np
import ml_dtypes
from concurrent.futures import ThreadPoolExecutor

N_CORES = 8
N_FULL, DIM, E = 32768, 2048, 64
NTOK = N_FULL // N_CORES  # tokens per core
P = 128
KCH = DIM // P            # dim chunks of 128
GROUP = 512               # tokens per matmul group (one PSUM bank col-half)
NGROUPS = NTOK // GROUP
TPG = GROUP // P          # 128-token tiles per group
HALF = 2048               # tokens per load unit
NH = NTOK // HALF
GPH = HALF // GROUP       # groups per half (4)
TOPK = 6

_cache = {}


def _build():
    if "nc" in _cache:
        return _cache["nc"]

    import concourse.mybir as mybir
    import concourse.tile as tile
    from concourse import bacc
    from concourse.masks import make_identity
    from concourse.vector_clock import ScopedClock

    class LeanTailTC(tile.TileContext):
        """TileContext with a single-execution kernel tail: drain + one
        all-engine barrier, skipping the semaphore clear pass and second
        barrier (the NEFF is executed once per load; semaphores are
        re-initialized by the runtime preamble on each load)."""

        def _drain_and_barrier(self, tick_clock, wait_clock):
            drain_inst = self.nc.sync.drain()
            wait_clock.add_sem_waits(
                drain_inst.ins, ScopedClock({None: tick_clock.global_clock})
            )
            popped = self.nc._tile_sem_poison_stack.pop()
            assert popped is self._sem_poison

    f32 = mybir.dt.float32
    f16 = mybir.dt.float16
    u32 = mybir.dt.uint32

    nc = bacc.Bacc(
        "TRN2",
        target_bir_lowering=False,
        debug=False,
        enable_asserts=False,
        num_devices=N_CORES,
    )
    xt_d = nc.dram_tensor("xt", [DIM, NTOK], f16, kind="ExternalInput").ap()
    wt_d = nc.dram_tensor("wt", [KCH, P, E], f16, kind="ExternalInput").ap()
    # partition-major output (one big-descriptor DMA); host reorders
    o_d = nc.dram_tensor(
        "o", [P, NGROUPS * TPG, 16], f32, kind="ExternalOutput"
    ).ap()

    with LeanTailTC(nc) as tc:
        with (
            tc.tile_pool(name="const", bufs=1) as cpool,
            tc.tile_pool(name="xt", bufs=2 * KCH) as xt_pool,
            tc.tile_pool(name="small", bufs=3) as sm_pool,
            tc.tile_pool(name="acc_ps", bufs=4, space="PSUM") as acc_pool,
            tc.tile_pool(name="lt_ps", bufs=2, space="PSUM") as ltp_pool,
        ):
            ident = cpool.tile([E, E], f32)
            make_identity(nc, ident)
            wt_sb = cpool.tile([P, KCH, E], f16)
            nc.scalar.dma_start(wt_sb, wt_d.rearrange("k p e -> p k e"))
            # all groups' results; single partition-major output DMA at the end
            wi_all = cpool.tile([P, NGROUPS * TPG, 16], f32)

            for h in range(NH):
                xts = []
                for k in range(KCH):
                    xk = xt_pool.tile([P, HALF], f16, tag="xt")
                    nc.sync.dma_start(
                        xk, xt_d[k * P : (k + 1) * P, h * HALF : (h + 1) * HALF]
                    )
                    xts.append(xk)

                # logits^T [64, 512] per group; 2 groups packed per PSUM bank
                accs = [
                    acc_pool.tile([P, GROUP], f32, tag="acc", name=f"acc_{h}_{a}")
                    for a in range(2)
                ]
                for k in range(KCH):
                    for j in range(GPH):
                        out_ap = accs[j // 2][(j % 2) * E : (j % 2 + 1) * E, :]
                        nc.tensor.matmul(
                            out_ap,
                            wt_sb[:, k, :],
                            xts[k][:, j * GROUP : (j + 1) * GROUP],
                            start=(k == 0),
                            stop=(k == KCH - 1),
                        )

                for j in range(GPH):
                    g = h * GPH + j
                    acc = accs[j // 2][(j % 2) * E : (j % 2 + 1) * E, :]

                    # back to token-major [P, TPG, E]
                    ltT = sm_pool.tile([E, GROUP], f32, tag="ltT")
                    nc.scalar.copy(out=ltT, in_=acc)
                    ltp = ltp_pool.tile([P, TPG, E], f32, tag="ltp")
                    for t in range(TPG):
                        nc.tensor.transpose(
                            ltp[:, t, :], ltT[:, t * P : (t + 1) * P], ident
                        )

                    # softmax + top-8 (top-6 taken on host); exp reads PSUM
                    e_sb = sm_pool.tile([P, TPG, E], f32, tag="esb")
                    nc.scalar.activation(e_sb, ltp, mybir.ActivationFunctionType.Exp)
                    s4 = sm_pool.tile([P, TPG], f32, tag="s4")
                    nc.vector.tensor_reduce(
                        s4, e_sb, axis=mybir.AxisListType.X, op=mybir.AluOpType.add
                    )
                    r4 = sm_pool.tile([P, TPG], f32, tag="r4")
                    nc.vector.reciprocal(r4, s4)

                    # wi[:, :, :8] = top-8 values, wi[:, :, 8:] = indices (u32 bits)
                    wi = wi_all[:, g * TPG : (g + 1) * TPG, :]
                    for t in range(TPG):
                        nc.vector.max(out=wi[:, t, 0:8], in_=e_sb[:, t, :])
                        nc.vector.max_index(
                            out=wi[:, t, 8:16].bitcast(u32),
                            in_max=wi[:, t, 0:8],
                            in_values=e_sb[:, t, :],
                        )
                    nc.vector.tensor_tensor(
                        wi[:, :, 0:8],
                        wi[:, :, 0:8],
                        r4[:, :, None].to_broadcast([P, TPG, 8]),
                        mybir.AluOpType.mult,
                    )

            nc.scalar.dma_start(o_d, wi_all)

    nc.compile()
    _cache["nc"] = nc
    return nc


last_results = None  # BassKernelResults of the most recent run (for test harness)


def _prep_shard(s):
    # [4096, 2048] f32 -> xT [2048, 4096] f16, contiguous
    return np.ascontiguousarray(np.asarray(s).T.astype(np.float16))


def kernel(x, weight):
    global last_results
    nc = _build()
    from concourse import bass_utils

    x = np.asarray(x)
    w = np.asarray(weight, dtype=np.float32)
    wt = np.ascontiguousarray(w.T).reshape(KCH, P, E).astype(np.float16)

    shards = [x[i * NTOK : (i + 1) * NTOK] for i in range(N_CORES)]
    with ThreadPoolExecutor(max_workers=N_CORES) as ex:
        xts = list(ex.map(_prep_shard, shards))

    in_maps = [{"xt": xts[i], "wt": wt} for i in range(N_CORES)]
    res = bass_utils.run_bass_kernel_spmd(nc, in_maps, core_ids=list(range(N_CORES)))
    last_results = res

    w_parts, i_parts = [], []
    for r in res.results:
        o = r["o"]  # [P, NGROUPS*TPG, 16] f32, partition-major
        o = np.ascontiguousarray(o.transpose(1, 0, 2)).reshape(NTOK, 16)
        w_parts.append(o[:, :TOPK])
        i_parts.append(o.view(np.uint32)[:, 8 : 8 + TOPK].astype(np.int32))
    return (
        np.ascontiguousarray(np.concatenate(w_parts, axis=0)),
        np.ascontiguousarray(np.concatenate(i_parts, axis=0)),
    )


# revision 39
# speedup vs baseline: 1.1398x; 1.0087x over previous
"""MoE gate routing (nn_Gate) on 8 Trainium2 NeuronCores via Bass/Tile.

x: [32768, 2048] f32, weight: [64, 2048] f32.
Returns (weights [32768, 6] f32, indices [32768, 6] i32) matching
softmax(x @ W^T) -> top-6 (values sorted descending).

Sharding: x split along the token dim across 8 cores (data parallel);
the tiny gate weight is replicated.  Host-side prep (part of the
sharding strategy): each shard is cast to fp16 and laid out dim-major
(xT [2048, 4096]) so the device streams it straight into the matmul
moving operand — no on-device transposes of x, and half the HBM bytes.
fp16 keeps 11 mantissa bits (x is N(0,1), well within range); logit
error is ~1e-4, so the top-6 softmax weights land at ~3e-4 relative
error, far under the 2e-2 gate.  Per core (~72 us, HBM-stream bound):
  - 32 x 512KB DMAs of xT chunk tiles [128 dim, 2048 tok] (4KB rows,
    sequential HBM, sync ring only; the whole shard resides in SBUF so
    the stream never stalls on buffer reuse)
  - W-stationary fp16 matmuls (moving N=512, 1 cycle/col),
    [64, 512] f32 PSUM accumulation over 16 dim-chunks (logits^T),
    two token-groups packed per PSUM bank via col tile_position
  - per 512-token group: PE transpose back to token-major, one ACT exp
    reading PSUM, DVE reduce+reciprocal for the softmax sum, DVE
    max8/max_index8 top-k, in-place scale
  - one partition-major [128, 32, 16] output DMA (2KB descriptors);
    the host reorders to token-major and splits values/indices
"""

import numpy as np
from concurrent.futures import ThreadPoolExecutor

N_CORES = 8
N_FULL, DIM, E = 32768, 2048, 64
NTOK = N_FULL // N_CORES  # tokens per core
P = 128
KCH = DIM // P            # dim chunks of 128
GROUP = 512               # tokens per matmul group (one PSUM bank col-half)
NGROUPS = NTOK // GROUP
TPG = GROUP // P          # 128-token tiles per group
HALF = 2048               # tokens per load unit
NH = NTOK // HALF
GPH = HALF // GROUP       # groups per half (4)
TOPK = 6

_cache = {}


def _build():
    if "nc" in _cache:
        return _cache["nc"]

    import concourse.mybir as mybir
    import concourse.tile as tile
    from concourse import bacc
    from concourse.masks import make_identity
    from concourse.vector_clock import ScopedClock

    class LeanTailTC(tile.TileContext):
        """TileContext with a lean kernel tail: only the final sync-engine
        drain (which waits for all DMA completions, so outputs are landed),
        skipping the semaphore clear pass and the two all-engine barriers.
        Safe because each NEFF load re-initializes semaphores; verified
        deterministic across repeated executions in one process."""

        def _drain_and_barrier(self, tick_clock, wait_clock):
            drain_inst = self.nc.sync.drain()
            wait_clock.add_sem_waits(
                drain_inst.ins, ScopedClock({None: tick_clock.global_clock})
            )
            popped = self.nc._tile_sem_poison_stack.pop()
            assert popped is self._sem_poison

    f32 = mybir.dt.float32
    f16 = mybir.dt.float16
    u32 = mybir.dt.uint32

    nc = bacc.Bacc(
        "TRN2",
        target_bir_lowering=False,
        debug=False,
        enable_asserts=False,
        num_devices=N_CORES,
    )
    xt_d = nc.dram_tensor("xt", [DIM, NTOK], f16, kind="ExternalInput").ap()
    wt_d = nc.dram_tensor("wt", [KCH, P, E], f16, kind="ExternalInput").ap()
    # partition-major output (one big-descriptor DMA); host reorders
    o_d = nc.dram_tensor(
        "o", [P, NGROUPS * TPG, 16], f32, kind="ExternalOutput"
    ).ap()

    with LeanTailTC(nc) as tc:
        with (
            tc.tile_pool(name="const", bufs=1) as cpool,
            tc.tile_pool(name="xt", bufs=2 * KCH) as xt_pool,
            tc.tile_pool(name="small", bufs=3) as sm_pool,
            tc.tile_pool(name="acc_ps", bufs=4, space="PSUM") as acc_pool,
            tc.tile_pool(name="lt_ps", bufs=2, space="PSUM") as ltp_pool,
        ):
            ident = cpool.tile([E, E], f32)
            make_identity(nc, ident)
            wt_sb = cpool.tile([P, KCH, E], f16)
            nc.scalar.dma_start(wt_sb, wt_d.rearrange("k p e -> p k e"))
            # all groups' results; single partition-major output DMA at the end
            wi_all = cpool.tile([P, NGROUPS * TPG, 16], f32)

            for h in range(NH):
                xts = []
                for k in range(KCH):
                    xk = xt_pool.tile([P, HALF], f16, tag="xt")
                    nc.sync.dma_start(
                        xk, xt_d[k * P : (k + 1) * P, h * HALF : (h + 1) * HALF]
                    )
                    xts.append(xk)

                # logits^T [64, 512] per group; 2 groups packed per PSUM bank
                accs = [
                    acc_pool.tile([P, GROUP], f32, tag="acc", name=f"acc_{h}_{a}")
                    for a in range(2)
                ]
                for k in range(KCH):
                    for j in range(GPH):
                        out_ap = accs[j // 2][(j % 2) * E : (j % 2 + 1) * E, :]
                        nc.tensor.matmul(
                            out_ap,
                            wt_sb[:, k, :],
                            xts[k][:, j * GROUP : (j + 1) * GROUP],
                            start=(k == 0),
                            stop=(k == KCH - 1),
                        )

                for j in range(GPH):
                    g = h * GPH + j
                    acc = accs[j // 2][(j % 2) * E : (j % 2 + 1) * E, :]

                    # back to token-major [P, TPG, E]
                    ltT = sm_pool.tile([E, GROUP], f32, tag="ltT")
                    nc.scalar.copy(out=ltT, in_=acc)
                    ltp = ltp_pool.tile([P, TPG, E], f32, tag="ltp")
                    for t in range(TPG):
                        nc.tensor.transpose(
                            ltp[:, t, :], ltT[:, t * P : (t + 1) * P], ident
                        )

                    # softmax + top-8 (top-6 taken on host); exp reads PSUM
                    e_sb = sm_pool.tile([P, TPG, E], f32, tag="esb")
                    nc.scalar.activation(e_sb, ltp, mybir.ActivationFunctionType.Exp)
                    s4 = sm_pool.tile([P, TPG], f32, tag="s4")
                    nc.vector.tensor_reduce(
                        s4, e_sb, axis=mybir.AxisListType.X, op=mybir.AluOpType.add
                    )
                    r4 = sm_pool.tile([P, TPG], f32, tag="r4")
                    nc.vector.reciprocal(r4, s4)

                    # wi[:, :, :8] = top-8 values, wi[:, :, 8:] = indices (u32 bits)
                    wi = wi_all[:, g * TPG : (g + 1) * TPG, :]
                    for t in range(TPG):
                        nc.vector.max(out=wi[:, t, 0:8], in_=e_sb[:, t, :])
                        nc.vector.max_index(
                            out=wi[:, t, 8:16].bitcast(u32),
                            in_max=wi[:, t, 0:8],
                            in_values=e_sb[:, t, :],
                        )
                    nc.vector.tensor_tensor(
                        wi[:, :, 0:8],
                        wi[:, :, 0:8],
                        r4[:, :, None].to_broadcast([P, TPG, 8]),
                        mybir.AluOpType.mult,
                    )

            nc.scalar.dma_start(o_d, wi_all)

    nc.compile()
    _cache["nc"] = nc
    return nc


last_results = None  # BassKernelResults of the most recent run (for test harness)


def _prep_shard(s):
    # [4096, 2048] f32 -> xT [2048, 4096] f16, contiguous
    return np.ascontiguousarray(np.asarray(s).T.astype(np.float16))


def kernel(x, weight):
    global last_results
    nc = _build()
    from concourse import bass_utils

    x = np.asarray(x)
    w = np.asarray(weight, dtype=np.float32)
    wt = np.ascontiguousarray(w.T).reshape(KCH, P, E).astype(np.float16)

    shards = [x[i * NTOK : (i + 1) * NTOK] for i in range(N_CORES)]
    with ThreadPoolExecutor(max_workers=N_CORES) as ex:
        xts = list(ex.map(_prep_shard, shards))

    in_maps = [{"xt": xts[i], "wt": wt} for i in range(N_CORES)]
    res = bass_utils.run_bass_kernel_spmd(nc, in_maps, core_ids=list(range(N_CORES)))
    last_results = res

    w_parts, i_parts = [], []
    for r in res.results:
        o = r["o"]  # [P, NGROUPS*TPG, 16] f32, partition-major
        o = np.ascontiguousarray(o.transpose(1, 0, 2)).reshape(NTOK, 16)
        w_parts.append(o[:, :TOPK])
        i_parts.append(o.view(np.uint32)[:, 8 : 8 + TOPK].astype(np.int32))
    return (
        np.ascontiguousarray(np.concatenate(w_parts, axis=0)),
        np.ascontiguousarray(np.concatenate(i_parts, axis=0)),
    )
